# revision 48
# baseline (speedup 1.0000x reference)
"""LCAOConv message-passing kernel for 8 Trainium2 NeuronCores.

Strategy (edge-parallel, owner = src core):
  - Node shard: core k owns nodes [k*NSH, (k+1)*NSH).
  - Phase A: each core computes h = MLP(x), c = MLP(coeffs) for its shard,
    writes a fused row table T[n] = [c[n] (R*D), h[n] (D)] in bf16, then
    AllGather -> full table on every core.
  - Phase B: edges are grouped by 128-node chunks of their src node.
    Per chunk (K tiles of 128 edge slots, one shot each):
      * one batched indirect-DMA gather of T[dst] rows (K*128 rows),
      * one-hot S_ne (node x edge) built from a DMA-broadcast srcl row via
        tensor_scalar is_equal (4x mode); S_en (edge x node) via
        tensor_tensor is_equal (2x, pair-duplicated operands),
      * c[src]+1 expanded by PE matmuls (bf16 out straight to PSUM),
      * per-edge reweighting in bf16 with every tensor_tensor in 2x mode
        (broadcast scalars are stored as duplicated pairs), halving trees
        for the d/r reductions, squares on the scalar engine,
      * segment-sum via PE matmuls accumulating into PSUM; agg @ Wu.
"""

import sys
for _p in ("/opt/trn_rl_repo", "/root/.axon_site/_ro/trn_rl_repo"):
    if _p not in sys.path:
        sys.path.insert(0, _p)

import numpy as np
import ml_dtypes

import concourse.bass as bass
import concourse.bacc as bacc
import concourse.mybir as mybir
import concourse.tile as tile
from concourse.bass import IndirectOffsetOnAxis
from concourse.bass_utils import run_bass_kernel_spmd

F32 = mybir.dt.float32
BF16 = mybir.dt.bfloat16
I32 = mybir.dt.int32
BF = ml_dtypes.bfloat16

NC = 8          # cores
P = 128         # partitions
CSP_B = 6       # expansion-matmul PSUM batch (groups per PSUM tile)

# CoreSim doesn't implement Silu; set True (sim checks only) to decompose
# silu(x) into x * sigmoid(x).
SIM_COMPAT_SILU = False

# Dump phase-B intermediates as extra outputs (debug only; small sizes).
DEBUG_DUMP = False


def _build(NSH, H, D, C, R, T_TILES, trace_enabled=False,
           no_collective=False, reps=1):
    """Build the Bass program (identical on all cores).

    reps > 1 repeats the whole computation in-program (timing use only —
    the per-rep time can then be resolved from wall-clock deltas despite
    the ~80ms axon dispatch overhead).
    """
    N = NSH * NC
    TD = R * D + D            # fused table row: c (R*D) + h (D)
    CD = R * D                # c part
    n_chunks = (NSH + P - 1) // P
    K = T_TILES               # tiles (128-edge groups) per chunk
    n_tiles = n_chunks * K
    KP = K * P                # edge slots per chunk
    KR = K * R

    nc = bacc.Bacc("TRN2", num_devices=NC)

    # ---- I/O ----
    xT = nc.dram_tensor("xT", [H, NSH], F32, kind="ExternalInput")
    cfT = nc.dram_tensor("cfT", [C, NSH * R], F32, kind="ExternalInput")
    W1 = nc.dram_tensor("W1", [H, H], F32, kind="ExternalInput")
    b1 = nc.dram_tensor("b1", [H, 1], F32, kind="ExternalInput")
    W2 = nc.dram_tensor("W2", [H, D], F32, kind="ExternalInput")
    b2r = nc.dram_tensor("b2r", [P, D], F32, kind="ExternalInput")
    Wc1 = nc.dram_tensor("Wc1", [C, H], F32, kind="ExternalInput")
    Wc2 = nc.dram_tensor("Wc2", [H, D], F32, kind="ExternalInput")
    Wu = nc.dram_tensor("Wu", [D, H], F32, kind="ExternalInput")
    dstI = nc.dram_tensor("dstI", [P, n_tiles], I32, kind="ExternalInput")
    srcl2 = nc.dram_tensor("srcl2", [P, n_tiles, 2], BF16,
                           kind="ExternalInput")
    rbf2 = nc.dram_tensor("rbf2", [P, n_tiles, R, 2], BF16,
                          kind="ExternalInput")
    srcR = nc.dram_tensor("srcR", [n_chunks, KP], BF16, kind="ExternalInput")
    out = nc.dram_tensor("out", [NSH, H], F32, kind="ExternalOutput")

    if DEBUG_DUMP:
        dbg = {
            "d_srcbc": nc.dram_tensor("d_srcbc", [n_chunks, P, KP], BF16,
                                      kind="ExternalOutput"),
            "d_sne": nc.dram_tensor("d_sne", [n_chunks, P, KP], BF16,
                                    kind="ExternalOutput"),
            "d_sall": nc.dram_tensor("d_sall", [n_chunks, P, K * P], BF16,
                                     kind="ExternalOutput"),
            "d_tdc": nc.dram_tensor("d_tdc", [n_chunks, P, K * CD], BF16,
                                    kind="ExternalOutput"),
            "d_tdh": nc.dram_tensor("d_tdh", [n_chunks, P, K * D], BF16,
                                    kind="ExternalOutput"),
            "d_ce": nc.dram_tensor("d_ce", [n_chunks, P, K * CD], BF16,
                                   kind="ExternalOutput"),
            "d_q5": nc.dram_tensor("d_q5", [n_chunks, P, KR], F32,
                                   kind="ExternalOutput"),
            "d_wv": nc.dram_tensor("d_wv", [n_chunks, P, K * D], BF16,
                                   kind="ExternalOutput"),
            "d_msg": nc.dram_tensor("d_msg", [n_chunks, P, K * D], BF16,
                                    kind="ExternalOutput"),
            "d_agg": nc.dram_tensor("d_agg", [n_chunks, D, P], F32,
                                    kind="ExternalOutput"),
        }

    # ---- internal DRAM ----
    # fused row table: [c (R*D) | h (D)] so one indirect gather per tile
    T_loc = nc.dram_tensor("T_loc", [NSH, TD], BF16, kind="Internal")
    T_full = nc.dram_tensor("T_full", [N, TD], BF16, kind="Internal",
                            addr_space="Shared")

    with tile.TileContext(nc) as tc:
        with tc.tile_pool(name="const", bufs=1) as cpool:
            # ---------- constants ----------
            W1_s = cpool.tile([H, H], F32)
            nc.sync.dma_start(W1_s[:], W1[:])
            b1_s = cpool.tile([H, 1], F32)
            nc.sync.dma_start(b1_s[:], b1[:])
            W2_s = cpool.tile([H, D], F32)
            nc.sync.dma_start(W2_s[:], W2[:])
            b2_s = cpool.tile([P, D], F32)
            nc.sync.dma_start(b2_s[:], b2r[:])
            # two stacked copies so matmuls can consume coeff tiles that
            # live on partitions [0,C) and [C,2C) (base partitions must match)
            Wc1_s = cpool.tile([2 * C, H], F32)
            nc.sync.dma_start(Wc1_s[:C, :], Wc1[:])
            nc.sync.dma_start(Wc1_s[C:2 * C, :], Wc1[:])
            Wc2_s = cpool.tile([H, D], F32)
            nc.sync.dma_start(Wc2_s[:], Wc2[:])
            Wu_s = cpool.tile([D, H], F32)
            nc.sync.dma_start(Wu_s[:], Wu[:])

            eps_s = cpool.tile([P, 1], F32)
            nc.vector.memset(eps_s[:], 1e-24)
            iota_p = cpool.tile([P, 1], F32)
            nc.gpsimd.iota(iota_p[:], pattern=[[0, 1]], base=0,
                           channel_multiplier=1,
                           allow_small_or_imprecise_dtypes=True)
            iota_row = cpool.tile([P, P], BF16)
            nc.gpsimd.iota(iota_row[:], pattern=[[1, P]], base=0,
                           channel_multiplier=0,
                           allow_small_or_imprecise_dtypes=True)

            # edge metadata, resident in SBUF
            dst_s = cpool.tile([P, n_tiles], I32)
            nc.sync.dma_start(dst_s[:], dstI[:])
            srcl2_s = cpool.tile([P, n_tiles, 2], BF16)
            nc.sync.dma_start(srcl2_s[:], srcl2[:])
            rbf2_s = cpool.tile([P, n_tiles, R, 2], BF16)
            nc.sync.dma_start(rbf2_s[:], rbf2[:])

            # ---------- phase A: node MLP -> H_loc ----------
            XW = 512

            def _silu(pool, dst_tag, in_ap, bias=None, shape=None):
                """silu into a fresh tile of `pool` (sim-compat aware)."""
                shape = shape or list(in_ap.shape)
                out_t = pool.tile(shape, F32, tag=dst_tag)
                o = out_t[tuple(slice(0, s) for s in in_ap.shape)]
                if not SIM_COMPAT_SILU:
                    if bias is None:
                        nc.scalar.activation(
                            o, in_ap, mybir.ActivationFunctionType.Silu)
                    else:
                        nc.scalar.activation(
                            o, in_ap, mybir.ActivationFunctionType.Silu,
                            bias=bias)
                else:
                    sg = pool.tile(shape, F32, tag=dst_tag + "_sg")
                    g = sg[tuple(slice(0, s) for s in in_ap.shape)]
                    if bias is None:
                        nc.scalar.activation(
                            g, in_ap, mybir.ActivationFunctionType.Sigmoid)
                        nc.vector.tensor_tensor(
                            out=o, in0=in_ap, in1=g,
                            op=mybir.AluOpType.mult)
                    else:
                        nc.scalar.activation(
                            g, in_ap, mybir.ActivationFunctionType.Sigmoid,
                            bias=bias)
                        xb = pool.tile(shape, F32, tag=dst_tag + "_xb")
                        x_ = xb[tuple(slice(0, s) for s in in_ap.shape)]
                        nc.vector.tensor_tensor(
                            out=x_, in0=in_ap,
                            in1=bias.to_broadcast(list(in_ap.shape)),
                            op=mybir.AluOpType.add)
                        nc.vector.tensor_tensor(
                            out=o, in0=x_, in1=g,
                            op=mybir.AluOpType.mult)
                return out_t
            with (
                tc.tile_pool(name="a_in", bufs=3) as a_in,
                tc.tile_pool(name="a_mid", bufs=2) as a_mid,
                tc.tile_pool(name="a_out", bufs=2) as a_out,
                tc.tile_pool(name="a_ps", bufs=2, space="PSUM") as a_ps,
                tc.tile_pool(name="a_ps2", bufs=1, space="PSUM") as a_ps2,
            ):
                nxt = (NSH + XW - 1) // XW
                for j in range(nxt):
                    w = min(XW, NSH - j * XW)
                    nb = (w + P - 1) // P
                    xt = a_in.tile([H, XW], F32, tag="xt")
                    nc.sync.dma_start(xt[:, :w], xT[:, j * XW:j * XW + w])
                    sx = _silu(a_mid, "sx", xt[:, :w], shape=[H, XW])
                    h1p = a_ps.tile([H, XW], F32, tag="h1p")
                    nc.tensor.matmul(h1p[:, :w], lhsT=W1_s[:],
                                     rhs=sx[:, :w],
                                     start=True, stop=True)
                    sh1 = _silu(a_mid, "sh1", h1p[:, :w], bias=b1_s[:],
                                shape=[H, XW])
                    h2p = a_ps2.tile([P, 4, D], F32, tag="h2p")
                    for b in range(nb):
                        bw = min(P, w - b * P)
                        nc.tensor.matmul(h2p[:bw, b, :],
                                         lhsT=sh1[:, b * P:b * P + bw],
                                         rhs=W2_s[:], start=True, stop=True)
                    h2r = a_out.tile([P, 4, D], BF16, tag="h2r")
                    nc.vector.tensor_tensor(
                        out=h2r[:, :nb, :], in0=h2p[:, :nb, :],
                        in1=b2_s[:].rearrange("p (o d) -> p o d", o=1
                                              ).to_broadcast([P, nb, D]),
                        op=mybir.AluOpType.add)
                    r0 = j * XW
                    if w == XW:
                        nc.sync.dma_start(
                            T_loc[r0:r0 + w, CD:TD].rearrange(
                                "(b p) d -> p b d", b=nb),
                            h2r[:, :nb, :])
                    else:
                        for b in range(nb):
                            bw = min(P, w - b * P)
                            nc.sync.dma_start(
                                T_loc[r0 + b * P:r0 + b * P + bw, CD:TD],
                                h2r[:bw, b, :])

                # ------ phase A: coeffs MLP -> T_loc c columns ------
                NR = NSH * R
                CW = 1024          # NR-cols per tile (128 nodes), 2-stacked
                nct = (NR + CW - 1) // CW
                for j in range(nct):
                    w = min(CW, NR - j * CW)
                    wa = min(XW, w)
                    wb = w - wa
                    ct = a_in.tile([2 * C, XW], F32, tag="ct")
                    nc.sync.dma_start(ct[:C, :wa],
                                      cfT[:, j * CW:j * CW + wa])
                    if wb > 0:
                        nc.sync.dma_start(
                            ct[C:2 * C, :wb],
                            cfT[:, j * CW + wa:j * CW + wa + wb])
                    if wb == wa:
                        sct = _silu(a_mid, "sct", ct[:], shape=[2 * C, XW])
                    else:
                        # partial final tile (never hit by the sim checks)
                        sct = a_mid.tile([2 * C, XW], F32, tag="sct")
                        nc.scalar.activation(
                            sct[:C, :wa], ct[:C, :wa],
                            mybir.ActivationFunctionType.Silu)
                        if wb > 0:
                            nc.scalar.activation(
                                sct[C:2 * C, :wb], ct[C:2 * C, :wb],
                                mybir.ActivationFunctionType.Silu)
                    c1p = a_ps.tile([H, 2, XW], F32, tag="c1p")
                    nc.tensor.matmul(c1p[:, 0, :wa], lhsT=Wc1_s[:C, :],
                                     rhs=sct[:C, :wa], start=True, stop=True)
                    if wb > 0:
                        nc.tensor.matmul(c1p[:, 1, :wb],
                                         lhsT=Wc1_s[C:2 * C, :],
                                         rhs=sct[C:2 * C, :wb],
                                         start=True, stop=True)
                    sc1a = _silu(a_mid, "sc1a", c1p[:, 0, :wa],
                                 shape=[H, XW])
                    sc1b = None
                    if wb > 0:
                        sc1b = _silu(a_mid, "sc1b", c1p[:, 1, :wb],
                                     shape=[H, XW])
                    c2p = a_ps2.tile([P, 8, D], F32, tag="c2p")
                    for b in range((w + P - 1) // P):
                        bw = min(P, w - b * P)
                        half, off = divmod(b * P, XW)
                        sc1h = sc1a if half == 0 else sc1b
                        nc.tensor.matmul(
                            c2p[:bw, b, :],
                            lhsT=sc1h[:, off:off + bw],
                            rhs=Wc2_s[:], start=True, stop=True)
                    c2r = a_out.tile([P, 8, D], BF16, tag="c2r")
                    nc.scalar.copy(c2r[:], c2p[:])
                    # per-128-block stores (T rows are 288 wide so the
                    # batched (b p) split would need a 4-dim DMA AP)
                    for b in range((w + P - 1) // P):
                        bw = min(P, w - b * P)
                        nr0 = j * CW + b * P
                        assert bw % R == 0
                        nc.sync.dma_start(
                            T_loc[nr0 // R:nr0 // R + bw // R,
                                  0:CD].rearrange(
                                "n (r d) -> n r d", d=D),
                            c2r[:bw, b, :])

            # ---------- AllGather the table ----------
            if no_collective:
                for k in range(NC):
                    nc.sync.dma_start(T_full[k * NSH:(k + 1) * NSH, :],
                                      T_loc[:])
            else:
                nc.gpsimd.collective_compute(
                    "AllGather",
                    mybir.AluOpType.bypass,
                    replica_groups=[list(range(NC))],
                    ins=[T_loc[:]],
                    outs=[T_full[:]],
                )

            # ---------- phase B: edges ----------
            with (
                tc.tile_pool(name="b_gat", bufs=2) as b_gat,
                tc.tile_pool(name="b_oh", bufs=2) as b_oh,
                tc.tile_pool(name="b_ce", bufs=2) as b_ce,
                tc.tile_pool(name="b_q", bufs=2) as b_q,
                tc.tile_pool(name="b_w", bufs=2) as b_w,
                tc.tile_pool(name="b_out", bufs=2) as b_out,
                tc.tile_pool(name="b_ps_e", bufs=2, space="PSUM") as b_ps_e,
                tc.tile_pool(name="b_ps_a", bufs=1, space="PSUM") as b_ps_a,
            ):
                SQ = mybir.ActivationFunctionType.Square
                SQRT = mybir.ActivationFunctionType.Sqrt
                for ch in range(n_chunks):
                    wn = min(P, NSH - ch * P)
                    t0 = ch * K

                    # indirect gathers of T[dst]: the HW SWDGE path only
                    # honors [P, 1] offsets (one row per partition), so one
                    # instruction per 128-edge tile
                    td = b_gat.tile([P, K, TD], BF16, tag="td")
                    for k in range(K):
                        nc.gpsimd.indirect_dma_start(
                            out=td[:, k, :],
                            out_offset=None,
                            in_=T_full[:],
                            in_offset=IndirectOffsetOnAxis(
                                ap=dst_s[:, t0 + k:t0 + k + 1], axis=0),
                        )

                    # local c block for this chunk (+1 pre-added)
                    cloc = b_oh.tile([P, CD], BF16, tag="cloc")
                    nc.sync.dma_start(cloc[:wn, :],
                                      T_loc[ch * P:ch * P + wn, 0:CD])
                    cp1 = b_oh.tile([P, CD], BF16, tag="cp1")
                    if wn < P:
                        # the expansion matmul contracts all 128 rows; HW
                        # SBUF garbage in the tail rows would turn 0*NaN
                        # into NaN (partition ranges must start aligned, so
                        # clear the whole tile then fill the valid rows)
                        nc.vector.memset(cp1[:], 0.0)
                    nc.vector.tensor_scalar_add(cp1[:wn, :], cloc[:wn, :],
                                                1.0)

                    # one-hots: S_ne (node x edge) and S_en (edge x node)
                    srcl_bc = b_oh.tile([P, KP], BF16, tag="srcl_bc")
                    nc.sync.dma_start(
                        srcl_bc[:], srcR[ch:ch + 1, :].to_broadcast([P, KP]))
                    sne = b_oh.tile([P, KP], BF16, tag="sne")
                    nc.vector.tensor_scalar(
                        out=sne[:], in0=srcl_bc[:], scalar1=iota_p[:],
                        scalar2=None, op0=mybir.AluOpType.is_equal)
                    s_all = b_oh.tile([P, K, P], BF16, tag="s_all")
                    nc.vector.tensor_tensor(
                        out=s_all[:].rearrange("p k (j two) -> p k j two",
                                               two=2),
                        in0=srcl2_s[:, t0:t0 + K, :].rearrange(
                            "p k (o two) -> p k o two", two=2
                        ).to_broadcast([P, K, P // 2, 2]),
                        in1=iota_row[:].rearrange(
                            "p (o j two) -> p o j two", o=1, two=2
                        ).to_broadcast([P, K, P // 2, 2]),
                        op=mybir.AluOpType.is_equal)

                    # expand (c[src]+1) via PE into f32 PSUM
                    ce = b_ce.tile([P, K, CD], BF16, tag="ce")
                    for g0 in range(0, K, CSP_B):
                        gb = min(CSP_B, K - g0)
                        csp = b_ps_e.tile([P, CSP_B, CD], F32, tag="csp")
                        for g in range(gb):
                            nc.tensor.matmul(
                                csp[:, g, :],
                                lhsT=sne[:, (g0 + g) * P:(g0 + g + 1) * P],
                                rhs=cp1[:], start=True, stop=True)
                        # ce = c[dst] * (c[src]+1)
                        nc.vector.tensor_tensor(
                            out=ce[:, g0:g0 + gb, :],
                            in0=csp[:, :gb, :],
                            in1=td[:, g0:g0 + gb, 0:CD],
                            op=mybir.AluOpType.mult)

                    # q2 = sum_d ce^2 per (edge, r): square on ACT + tree
                    sq = b_ce.tile([P, K, CD], BF16, tag="sq")
                    nc.scalar.activation(sq[:], ce[:], SQ)
                    sqv = sq[:].rearrange("p k (r d) -> p (k r) d", d=D)
                    q1 = b_q.tile([P, KR, 16], BF16, tag="q1")
                    nc.vector.tensor_add(q1[:], sqv[:, :, 0:16],
                                         sqv[:, :, 16:32])
                    q2t = b_q.tile([P, KR, 8], BF16, tag="q2t")
                    nc.vector.tensor_add(q2t[:], q1[:, :, 0:8], q1[:, :, 8:16])
                    q3 = b_q.tile([P, KR, 4], BF16, tag="q3")
                    nc.vector.tensor_add(q3[:], q2t[:, :, 0:4], q2t[:, :, 4:8])
                    q4 = b_q.tile([P, KR, 2], BF16, tag="q4")
                    nc.vector.tensor_add(q4[:], q3[:, :, 0:2], q3[:, :, 2:4])
                    q5 = b_q.tile([P, KR], F32, tag="q5")
                    nc.vector.tensor_add(q5[:], q4[:, :, 0], q4[:, :, 1])
                    dq = b_q.tile([P, KR], F32, tag="dq")
                    nc.scalar.activation(dq[:], q5[:], SQRT, bias=eps_s[:])
                    rq = b_q.tile([P, KR], F32, tag="rq")
                    nc.vector.reciprocal(rq[:], dq[:])
                    # s_w2[p, kr, 2] = (rbf / q) duplicated pairs
                    s_w2 = b_q.tile([P, KR, 2], BF16, tag="s_w2")
                    nc.vector.tensor_tensor(
                        out=s_w2[:],
                        in0=rq[:].rearrange("p (kr o) -> p kr o", o=1
                                            ).to_broadcast([P, KR, 2]),
                        in1=rbf2_s[:, t0:t0 + K, :, :].rearrange(
                            "p k r two -> p (k r) two"),
                        op=mybir.AluOpType.mult)

                    # sce = ce * s_w (pair-broadcast, 2x) then r-tree
                    sce = b_ce.tile([P, K, CD], BF16, tag="sce")
                    nc.vector.tensor_tensor(
                        out=sce[:].rearrange("p k (r h two) -> p (k r) h two",
                                             two=2, h=D // 2),
                        in0=ce[:].rearrange("p k (r h two) -> p (k r) h two",
                                            two=2, h=D // 2),
                        in1=s_w2[:].rearrange("p kr (o two) -> p kr o two",
                                              two=2
                                              ).to_broadcast(
                            [P, KR, D // 2, 2]),
                        op=mybir.AluOpType.mult)
                    scev = sce[:].rearrange("p k (r d) -> p k r d", d=D)
                    t1 = b_w.tile([P, K, 4, D], BF16, tag="t1")
                    nc.vector.tensor_add(t1[:], scev[:, :, 0:4, :],
                                         scev[:, :, 4:8, :])
                    t2 = b_w.tile([P, K, 2, D], BF16, tag="t2")
                    nc.vector.tensor_add(t2[:], t1[:, :, 0:2, :],
                                         t1[:, :, 2:4, :])
                    wv = b_w.tile([P, K, D], BF16, tag="wv")
                    nc.vector.tensor_add(wv[:], t2[:, :, 0, :], t2[:, :, 1, :])

                    # second l2norm over d
                    wsq = b_w.tile([P, K, D], BF16, tag="wsq")
                    nc.vector.tensor_tensor(out=wsq[:], in0=wv[:], in1=wv[:],
                                            op=mybir.AluOpType.mult)
                    ws = b_w.tile([P, K], F32, tag="ws")
                    nc.vector.reduce_sum(ws[:], wsq[:],
                                         axis=mybir.AxisListType.X)
                    dw = b_w.tile([P, K], F32, tag="dw")
                    nc.scalar.activation(dw[:], ws[:], SQRT, bias=eps_s[:])
                    rw = b_w.tile([P, K], F32, tag="rw")
                    nc.vector.reciprocal(rw[:], dw[:])
                    rw2 = b_w.tile([P, K, 2], BF16, tag="rw2")
                    nc.vector.tensor_scalar_mul(
                        rw2[:],
                        rw[:].rearrange("p (k o) -> p k o", o=1
                                        ).to_broadcast([P, K, 2]),
                        1.0)

                    # msg = h[dst] * wv * rw
                    m1 = b_w.tile([P, K, D], BF16, tag="m1")
                    nc.vector.tensor_tensor(out=m1[:], in0=wv[:],
                                            in1=td[:, :, CD:TD],
                                            op=mybir.AluOpType.mult)
                    msg = b_w.tile([P, K, D], BF16, tag="msg")
                    nc.vector.tensor_tensor(
                        out=msg[:].rearrange("p k (h two) -> p k h two",
                                             two=2),
                        in0=m1[:].rearrange("p k (h two) -> p k h two", two=2),
                        in1=rw2[:].rearrange("p k (o two) -> p k o two",
                                             two=2
                                             ).to_broadcast(
                            [P, K, D // 2, 2]),
                        op=mybir.AluOpType.mult)

                    # segment-sum into agg^T via PE
                    aggp = b_ps_a.tile([D, P], F32, tag="aggp")
                    for k in range(K):
                        nc.tensor.matmul(
                            aggp[:], lhsT=msg[:, k, :], rhs=s_all[:, k, :],
                            start=(k == 0), stop=(k == K - 1))

                    # chunk tail: out rows = agg @ Wu
                    aggs = b_out.tile([D, P], F32, tag="aggs")
                    nc.scalar.copy(aggs[:], aggp[:])
                    outp = b_ps_a.tile([P, H], F32, tag="outp")
                    nc.tensor.matmul(outp[:wn, :], lhsT=aggs[:, :wn],
                                     rhs=Wu_s[:], start=True, stop=True)
                    outs = b_out.tile([P, H], F32, tag="outs")
                    nc.scalar.copy(outs[:wn, :], outp[:wn, :])
                    nc.sync.dma_start(out[ch * P:ch * P + wn, :],
                                      outs[:wn, :])

                    if DEBUG_DUMP:
                        nc.sync.dma_start(dbg["d_srcbc"][ch], srcl_bc[:])
                        nc.sync.dma_start(dbg["d_sne"][ch], sne[:])
                        nc.sync.dma_start(
                            dbg["d_sall"][ch],
                            s_all[:].rearrange("p k j -> p (k j)"))
                        nc.sync.dma_start(
                            dbg["d_tdc"][ch],
                            td[:, :, 0:CD].rearrange("p k c -> p (k c)"))
                        nc.sync.dma_start(
                            dbg["d_tdh"][ch],
                            td[:, :, CD:TD].rearrange("p k d -> p (k d)"))
                        nc.sync.dma_start(
                            dbg["d_ce"][ch],
                            ce[:].rearrange("p k c -> p (k c)"))
                        nc.sync.dma_start(dbg["d_q5"][ch], q5[:])
                        nc.sync.dma_start(
                            dbg["d_wv"][ch],
                            wv[:].rearrange("p k d -> p (k d)"))
                        nc.sync.dma_start(
                            dbg["d_msg"][ch],
                            msg[:].rearrange("p k d -> p (k d)"))
                        nc.sync.dma_start(dbg["d_agg"][ch], aggs[:])

    nc.finalize()
    return nc


def _prepare(inputs, NSH, H, D, C, R):
    """Host-side sharding: returns (in_maps, T_TILES)."""
    x = np.asarray(inputs["x"], np.float32)
    rbfs = np.asarray(inputs["rbfs"], np.float32)
    coeffs = np.asarray(inputs["coeffs"], np.float32)
    W1 = np.asarray(inputs["W1"], np.float32)
    b1 = np.asarray(inputs["b1"], np.float32)
    W2 = np.asarray(inputs["W2"], np.float32)
    b2 = np.asarray(inputs["b2"], np.float32)
    Wc1 = np.asarray(inputs["Wc1"], np.float32)
    Wc2 = np.asarray(inputs["Wc2"], np.float32)
    Wu = np.asarray(inputs["Wu"], np.float32)
    ei = np.asarray(inputs["edge_index"], np.int64)
    src, dst = ei[0], ei[1]
    N, E = x.shape[0], src.shape[0]
    n_chunks = (NSH + P - 1) // P

    core_of = src // NSH
    src_loc = src - core_of * NSH
    chunk = src_loc // P
    sic = src_loc % P          # src index within chunk

    # count edges per (core, chunk)
    cc = core_of * n_chunks + chunk
    counts = np.bincount(cc, minlength=NC * n_chunks)
    T_TILES = -(-int(counts.max()) // P)
    n_tiles = n_chunks * T_TILES
    slots_per_chunk = T_TILES * P

    # slot assignment: order edges by (core, chunk), sequential within
    order = np.argsort(cc, kind="stable")
    cc_sorted = cc[order]
    within = np.arange(E) - np.concatenate(
        ([0], np.cumsum(np.bincount(cc_sorted, minlength=NC * n_chunks))))[
        cc_sorted]
    slot = cc_sorted * slots_per_chunk + within

    dst_all = np.zeros((NC, n_tiles * P), np.int32)
    srcl_all = np.zeros((NC, n_tiles * P), np.float32)
    rbf_all = np.zeros((NC, n_tiles * P, R), np.float32)
    core_sorted = slot // (n_chunks * slots_per_chunk)
    slot_in_core = slot % (n_chunks * slots_per_chunk)
    dst_all[core_sorted, slot_in_core] = dst[order].astype(np.int32)
    srcl_all[core_sorted, slot_in_core] = sic[order].astype(np.float32)
    rbf_all[core_sorted, slot_in_core] = rbfs[order]

    in_maps = []
    for k in range(NC):
        lo, hi = k * NSH, (k + 1) * NSH
        # [P, n_tiles] layouts (slot = tile*128 + p)
        dstI = dst_all[k].reshape(n_tiles, P).T
        srcl = srcl_all[k].reshape(n_tiles, P).T          # [P, n_tiles]
        srcl2 = np.repeat(srcl[:, :, None], 2, axis=2)    # [P, n_tiles, 2]
        rbfk = rbf_all[k].reshape(n_tiles, P, R).transpose(1, 0, 2)
        rbf2 = np.repeat(rbfk[:, :, :, None], 2, axis=3)  # [P,n_tiles,R,2]
        # per-chunk row of src-within-chunk, e index = tile*128 + p
        srcR = srcl_all[k].reshape(n_chunks, T_TILES * P)
        in_maps.append({
            "xT": np.ascontiguousarray(x[lo:hi].T),
            "cfT": np.ascontiguousarray(
                coeffs[lo:hi].reshape(NSH * R, C).T),
            "W1": np.ascontiguousarray(W1),
            "b1": np.ascontiguousarray(b1.reshape(H, 1)),
            "W2": np.ascontiguousarray(W2),
            "b2r": np.ascontiguousarray(np.tile(b2, (P, 1))),
            "Wc1": np.ascontiguousarray(Wc1),
            "Wc2": np.ascontiguousarray(Wc2),
            "Wu": np.ascontiguousarray(Wu),
            "dstI": np.ascontiguousarray(dstI),
            "srcl2": np.ascontiguousarray(srcl2.astype(BF)),
            "rbf2": np.ascontiguousarray(rbf2.astype(BF)),
            "srcR": np.ascontiguousarray(srcR.astype(BF)),
        })
    return in_maps, T_TILES


_CACHE = {}


def _get_nc(inputs):
    x = np.asarray(inputs["x"])
    coeffs = np.asarray(inputs["coeffs"])
    N, H = x.shape
    _, R, C = coeffs.shape
    D = np.asarray(inputs["W2"]).shape[1]
    assert N % NC == 0
    NSH = N // NC
    in_maps, T_TILES = _prepare(inputs, NSH, H, D, C, R)
    key = (NSH, H, D, C, R, T_TILES)
    if key not in _CACHE:
        _CACHE[key] = _build(NSH, H, D, C, R, T_TILES)
    return _CACHE[key], in_maps


def run(inputs, trace=False):
    """Returns (output, BassKernelResults)."""
    nc, in_maps = _get_nc(inputs)
    res = run_bass_kernel_spmd(nc, in_maps, core_ids=list(range(NC)),
                               trace=trace)
    outs = [res.results[k]["out"] for k in range(NC)]
    return np.concatenate(outs, axis=0), res


def make_runner(inputs, reps=1):
    """Build a steady-state timed runner: jit once, device-resident inputs.

    The jitted call chains `reps` kernel executions back-to-back on device
    (each rep's outputs become the next rep's donated output operands), so
    per-kernel time can be resolved as a slope between two reps values
    despite the ~80ms axon dispatch overhead.
    """
    import jax
    from jax.sharding import Mesh, PartitionSpec
    from jax.experimental.shard_map import shard_map
    from concourse import bass2jax

    nc, in_maps = _get_nc(inputs)
    bass2jax.install_neuronx_cc_hook()

    partition_name = (nc.partition_id_tensor.name
                      if nc.partition_id_tensor else None)
    import concourse.mybir as mybir_
    in_names, out_names, out_avals, zero_outs = [], [], [], []
    for alloc in nc.m.functions[0].allocations:
        if not isinstance(alloc, mybir_.MemoryLocationSet):
            continue
        name = alloc.memorylocations[0].name
        if alloc.kind == "ExternalInput":
            if name != partition_name:
                in_names.append(name)
        elif alloc.kind == "ExternalOutput":
            shape = tuple(alloc.tensor_shape)
            dtype = mybir_.dt.np(alloc.dtype)
            out_names.append(name)
            out_avals.append(jax.core.ShapedArray(shape, dtype))
            zero_outs.append(np.zeros(shape, dtype))
    n_params = len(in_names)
    n_outs = len(out_avals)
    all_in_names = list(in_names) + out_names
    if partition_name is not None:
        all_in_names.append(partition_name)

    def _body(*args):
        ins = list(args[:n_params])
        outs = list(args[n_params:n_params + n_outs])
        for _ in range(reps):
            operands = ins + outs
            if partition_name is not None:
                operands.append(bass2jax.partition_id_tensor())
            outs = list(bass2jax._bass_exec_p.bind(
                *operands,
                out_avals=tuple(out_avals),
                in_names=tuple(all_in_names),
                out_names=tuple(out_names),
                lowering_input_output_aliases=(),
                sim_require_finite=True,
                sim_require_nnan=True,
                nc=nc,
            ))
        return tuple(outs)

    devices = jax.devices()[:NC]
    mesh = Mesh(np.asarray(devices), ("core",))
    in_specs = (PartitionSpec("core"),) * (n_params + n_outs)
    out_specs = (PartitionSpec("core"),) * len(out_names)
    donate = tuple(range(n_params, n_params + n_outs))
    sharded = jax.jit(
        shard_map(_body, mesh=mesh, in_specs=in_specs,
                  out_specs=out_specs, check_rep=False),
        donate_argnums=donate,
        keep_unused=True,
    )
    from jax.sharding import NamedSharding
    sh = NamedSharding(mesh, PartitionSpec("core"))
    concat_in = [
        jax.device_put(
            np.concatenate([np.asarray(in_maps[c][nm]) for c in range(NC)],
                           axis=0), sh)
        for nm in in_names]

    def _fresh_zeros():
        return [jax.device_put(
            np.zeros((NC * z.shape[0], *z.shape[1:]), z.dtype), sh)
            for z in zero_outs]

    oidx = out_names.index("out")
    osh = out_avals[oidx].shape

    def call(zeros=None):
        outs = sharded(*concat_in, *(zeros if zeros is not None
                                     else _fresh_zeros()))
        jax.block_until_ready(outs)
        return outs

    def unpack(outs):
        arr = np.asarray(outs[oidx]).reshape(NC, *osh)
        return np.concatenate([arr[c] for c in range(NC)], axis=0)

    return call, unpack, _fresh_zeros


def kernel(**inputs) -> np.ndarray:
    out, _ = run(inputs, trace=False)
    return out
